# revision 23
# baseline (speedup 1.0000x reference)
"""PointFlow2DVAE loss kernel for 8 Trainium2 NeuronCores.

Data-parallel over batch B=8: one point cloud per core. CNF layer-2 /
velocity / divergence matmuls run as fp8e4m3 DoubleRow (0.5 cyc/row);
layer-1 and chamfer distance matmuls are f32r (1 cyc/row, sim-exact).
tanh on Act, squares split DVE-TT / Pool-TT, Euler updates fused as
scalar_tensor_tensor with the dt scale. The divergence uses the identity
div = sum (1-h2^2)*(ru - Wu@h1sq) with ru folded into the ups matmul as a
K=1 ones-row accumulation, so no host-side correction terms are needed.
Chamfer: single D matrix per r-block, Act-engine bf16 copy, TT-min trees
(Pool does level 1), running min across blocks for the x-side.
"""
import numpy as np
import ml_dtypes

import concourse.bacc as bacc
import concourse.bass as bass
import concourse.tile as tile
from concourse import mybir
from concourse.bass_utils import run_bass_kernel_spmd

B, N, D = 8, 2048, 2
LAT, ENC_H, CNF_H = 128, 256, 256
STEPS = 10
DT = 1.0 / STEPS
LAM_R, LAM_P, LAM_E, LAM_C, LAM_V = 1.0, 0.1, 0.01, 10.0, 0.01
LOG2PI = float(np.log(2.0 * np.pi))
WU_SCALE = 1024.0

NT = 512
NNT = N // NT
import os
PHASES = os.environ.get("KPHASES", "enc,cnf,cham").split(",")

F32 = mybir.dt.float32
F32R = mybir.dt.float32r
BF16 = mybir.dt.bfloat16
FP8 = mybir.dt.float8e4
BF = ml_dtypes.bfloat16
F8 = ml_dtypes.float8_e4m3
AF = mybir.ActivationFunctionType
ALU = mybir.AluOpType
AX = mybir.AxisListType
DR = mybir.MatmulPerfMode.DoubleRow


def host_precompute(w):
    f = np.float32
    W1, b1, W2, b2, W3, b3 = w["W1"], w["b1"], w["W2"], w["b2"], w["W3"], w["b3"]
    pre = {}
    pre["enc1"] = np.ascontiguousarray(
        np.concatenate([w["We1"].T, w["be1"][None, :]], 0), f)       # [3,256]
    # [k, mb, kb, j] = W[mb*128+j, kb*128+k]
    def lhst4(Wm):
        return np.ascontiguousarray(
            Wm.reshape(2, 128, 2, 128).transpose(3, 0, 2, 1))
    pre["We2b"] = lhst4(w["We2"]).astype(BF)
    pre["W2f8"] = lhst4(W2).astype(F8)
    pre["WmuT"] = np.ascontiguousarray(
        w["Wmu"].T.reshape(2, 128, 128).transpose(1, 0, 2), f)
    pre["WlvT"] = np.ascontiguousarray(
        w["Wlv"].T.reshape(2, 128, 128).transpose(1, 0, 2), f)
    pre["bmulv"] = np.ascontiguousarray(
        np.stack([w["bmu"], w["blv"]], 1), f)

    W1p = W1[:, 0:2]
    W1t = W1[:, 2]
    pre["W1zT"] = np.ascontiguousarray(W1[:, 3:].T, f)
    pre["a1top"] = np.ascontiguousarray(
        np.broadcast_to(W1p.T.reshape(2, 1, 2, 128), (2, 20, 2, 128)), f)
    idx = np.arange(STEPS)
    pb3 = W1p @ b3
    TB_e = (idx * DT)[:, None] * W1t[None, :] + b1[None, :] \
        + (idx * DT)[:, None] * pb3[None, :]
    TB_g = (1.0 - idx * DT)[:, None] * W1t[None, :] + b1[None, :] \
        - (idx * DT)[:, None] * pb3[None, :]
    pre["TB"] = np.ascontiguousarray(np.concatenate([TB_e, TB_g], 0), f)

    # vW[j, kb, m] = W3[m, kb*128+j], padded to M=16 for dual-fp8 ldweights
    vW = np.zeros((128, 2, 16), np.float32)
    vW[:, :, 0:2] = W3.T.reshape(2, 128, 2).transpose(1, 0, 2)
    pre["vWf8"] = vW.astype(F8)

    c0, c1 = W1[:, 0], W1[:, 1]
    Wu = (W3[0][:, None] * W2) * c0[None, :] + (W3[1][:, None] * W2) * c1[None, :]
    pre["nWuf8"] = (lhst4(-Wu) * WU_SCALE).astype(F8)
    ru = Wu.sum(1)                                                   # [256]
    pre["ruf8"] = np.ascontiguousarray(
        (ru.reshape(2, 128)[None] * WU_SCALE)).astype(F8)            # [1,2,128]
    pre["onesf8"] = np.ones((1, N), F8)

    pre["identb"] = np.eye(128).astype(BF)
    pre["ones20"] = np.ones((1, 20), f)
    pre["TB0"] = np.ascontiguousarray(pre["TB"][0:1])
    pre["TB10"] = np.ascontiguousarray(pre["TB"][STEPS:STEPS + 1])
    pre["b3c"] = np.ascontiguousarray(b3[:, None], f)
    pre["nb3c"] = np.ascontiguousarray(-b3[:, None], f)
    return pre


WEIGHT_SPECS = [
    ("enc1", (3, 256), F32R),
    ("We2b", (128, 2, 2, 128), BF16), ("W2f8", (128, 2, 2, 128), FP8),
    ("WmuT", (128, 2, 128), F32), ("WlvT", (128, 2, 128), F32),
    ("bmulv", (128, 2), F32),
    ("W1zT", (128, 256), F32), ("a1top", (2, 20, 2, 128), F32R),
    ("TB", (20, 256), F32),
    ("vWf8", (128, 2, 16), FP8), ("nWuf8", (128, 2, 2, 128), FP8),
    ("ruf8", (1, 2, 128), FP8), ("onesf8", (1, N), FP8),
    ("identb", (128, 128), BF16),
    ("b3c", (2, 1), F32), ("nb3c", (2, 1), F32), ("ones20", (1, 20), F32),
    ("TB0", (1, 256), F32), ("TB10", (1, 256), F32),
]


def build_nc():
    nc = bacc.Bacc("TRN2", target_bir_lowering=False, debug=False,
                   enable_asserts=False, num_devices=B)
    ins = {}
    ins["xT3"] = nc.dram_tensor("xT3", [3, N], F32R, kind="ExternalInput").ap()
    ins["nT3"] = nc.dram_tensor("nT3", [3, N], F32R, kind="ExternalInput").ap()
    ins["epsc"] = nc.dram_tensor("epsc", [LAT, 1], F32, kind="ExternalInput").ap()
    for name, shape, dt_ in WEIGHT_SPECS:
        ins[name] = nc.dram_tensor(name, list(shape), dt_, kind="ExternalInput").ap()
    outs = {}
    for name, shape in [("o_div", [128]), ("o_mu", [128]), ("o_lv", [128]),
                        ("o_chA", [128]), ("o_chB", [128]), ("o_sy2", [2])]:
        outs[name] = nc.dram_tensor(name, shape, F32, kind="ExternalOutput").ap()

    with tile.TileContext(nc) as tc:
        _body(nc, tc, ins, outs)
    nc.compile()
    return nc


def _body(nc, tc, ins, outs):
    from contextlib import ExitStack
    with ExitStack() as ctx:
        const = ctx.enter_context(tc.tile_pool(name="const", bufs=1))
        state = ctx.enter_context(tc.tile_pool(name="state", bufs=1))
        work = ctx.enter_context(tc.tile_pool(name="work", bufs=2))
        small = ctx.enter_context(tc.tile_pool(name="small", bufs=1))

        stF = [state.tile([3, N], F32R, tag=f"stF{p}", name=f"stF{p}")
               for p in range(2)]
        stG = [state.tile([3, N], F32R, tag=f"stG{p}", name=f"stG{p}")
               for p in range(2)]
        ones_dram = ins["xT3"][2:3]
        c = {}
        EARLY = ["enc1", "We2b", "WmuT", "WlvT", "bmulv", "W1zT", "TB"]
        LATE = [s[0] for s in WEIGHT_SPECS if s[0] not in EARLY and s[0] != "a1top"]
        for name, shape, dt_ in WEIGHT_SPECS:
            if name == "a1top":
                continue
            c[name] = const.tile(list(shape), dt_, tag=name, name=f"c_{name}")
        nc.sync.dma_start(out=stF[0], in_=ins["xT3"])
        for name in EARLY:
            nc.sync.dma_start(out=c[name], in_=ins[name])
        eps_s = small.tile([LAT, 1], F32, tag="eps")
        nc.sync.dma_start(out=eps_s, in_=ins["epsc"])
        nc.sync.dma_start(out=stG[0], in_=ins["nT3"])
        nc.sync.dma_start(out=stF[1][2:3], in_=ins["xT3"][2:3])
        nc.sync.dma_start(out=stG[1][2:3], in_=ins["nT3"][2:3])
        ones1p = state.tile([1, N], F32R, tag="ones1p")
        nc.sync.dma_start(out=ones1p, in_=ones_dram)
        for name in LATE:
            nc.sync.dma_start(out=c[name], in_=ins[name])

        xnegA = state.tile([6, N], F32R, tag="xnegA", name="xnegA")
        raugA = state.tile([6, N], F32R, tag="raugA", name="raugA")
        sqx = work.tile([2, N], F32R, tag="sqx", bufs=1, name="sqx")
        nc.vector.scalar_tensor_tensor(out=sqx, in0=stF[0][0:2], scalar=1.0,
                                       in1=stF[0][0:2], op0=ALU.mult, op1=ALU.mult)
        negx = work.tile([2, N], F32R, tag="negx", bufs=1, name="negx")
        nc.vector.tensor_scalar(negx, stF[0][0:2], -2.0, None, ALU.mult)
        nc.sync.dma_start(out=xnegA[0:2], in_=negx)
        nc.sync.dma_start(out=xnegA[2:4], in_=ones_dram.partition_broadcast(2))
        nc.sync.dma_start(out=xnegA[4:6], in_=sqx)
        nc.sync.dma_start(out=raugA[4:6], in_=ones_dram.partition_broadcast(2))

        divslots = small.tile([128, STEPS * NNT], F32, tag="divslots")

        psE_cm = tc.tile_pool(name="psE", bufs=3, space="PSUM")
        psE = psE_cm.__enter__()

        # ================= encoder =================
        gparts = small.tile([128, 2, NNT], F32, tag="gparts")
        g_s = small.tile([128, 2], F32, tag="g")
        if "enc" in PHASES:
            # PE warmup (p-state ramp) on the first-loaded tile; output unused
            warm = psE.tile([128, 2, NT], F32, tag="e", bufs=4)
            for w in range(4):
                nc.tensor.matmul(warm[:, 0, :], stF[0][0:2, 0:128],
                                 stF[0][0:2, 0:NT], start=True, stop=True)
            sEs = []
            h1es = []
            for nt in range(NNT):
                sl = slice(nt * NT, (nt + 1) * NT)
                sE = psE.tile([128, 2, NT], F32, tag="e", bufs=4)
                for mb in range(2):
                    nc.tensor.matmul(sE[:, mb, :], c["enc1"][:, mb * 128:(mb + 1) * 128],
                                     stF[0][:, sl], start=True, stop=True)
                h1e = work.tile([128, 2, NT], BF16, tag="h1e", bufs=4)
                nc.scalar.activation(h1e, sE, AF.Relu)
                sEs.append(sE)
                h1es.append(h1e)
            for nt in range(NNT):
                sE, h1e = sEs[nt], h1es[nt]
                for mb in range(2):
                    for kb in range(2):
                        nc.tensor.matmul(sE[:, mb, :], c["We2b"][:, mb, kb, :],
                                         h1e[:, kb, :], start=(kb == 0), stop=(kb == 1))
                h2e = work.tile([128, 2, NT], BF16, tag="h1e", bufs=4)
                nc.scalar.activation(h2e, sE, AF.Relu)
                for mb in range(2):
                    nc.vector.tensor_reduce(gparts[:, mb, nt:nt + 1],
                                            h2e[:, mb, :], axis=AX.X, op=ALU.max)
            for mb in range(2):
                nc.vector.tensor_reduce(g_s[:, mb:mb + 1], gparts[:, mb, :],
                                        axis=AX.X, op=ALU.max)
            sM = psE.tile([128, 2, NT], F32, tag="e", bufs=4)
            sM2 = psE.tile([128, 2, NT], F32, tag="e", bufs=4)
            mu_ps = sM[:, 0, 0:1]
            lv_ps = sM[:, 1, 0:1]
            for kb in range(2):
                nc.tensor.matmul(mu_ps, c["WmuT"][:, kb, :], g_s[:, kb:kb + 1],
                                 start=(kb == 0), stop=(kb == 1))
            for kb in range(2):
                nc.tensor.matmul(lv_ps, c["WlvT"][:, kb, :], g_s[:, kb:kb + 1],
                                 start=(kb == 0), stop=(kb == 1))
            mu_s = small.tile([128, 1], F32, tag="mu_s")
            lv_s = small.tile([128, 1], F32, tag="lv_s")
            nc.vector.tensor_scalar(mu_s, mu_ps, c["bmulv"][:, 0:1], None, ALU.add)
            nc.vector.tensor_scalar(lv_s, lv_ps, c["bmulv"][:, 1:2], None, ALU.add)
            nc.sync.dma_start(out=outs["o_mu"], in_=mu_s)
            nc.sync.dma_start(out=outs["o_lv"], in_=lv_s)
            e_s = small.tile([128, 1], F32, tag="e_s")
            nc.scalar.activation(e_s, lv_ps, AF.Exp, scale=0.5)
            z_s = small.tile([128, 1], F32, tag="z_s")
            nc.vector.tensor_tensor(z_s, e_s, eps_s, ALU.mult)
            nc.vector.tensor_tensor(z_s, z_s, mu_ps, ALU.add)
            cz_ps = sM2[0:1, 0, 0:256]
            nc.tensor.matmul(cz_ps, z_s, c["W1zT"], start=True, stop=True)
            czrow_s = small.tile([1, 256], F32, tag="czrow_s")
            nc.vector.tensor_copy(czrow_s, cz_ps)
            sM3 = psE.tile([128, 2, NT], F32, tag="e", bufs=4)
            czb = sM3[0:20, 0, 0:256]
            nc.tensor.matmul(czb, c["ones20"], czrow_s, start=True, stop=True)
            brows = state.tile([20, 256], F32R, tag="brows")
            brows0 = state.tile([1, 256], F32R, tag="brows0")
            brows10 = state.tile([1, 256], F32R, tag="brows10")
            nc.vector.tensor_tensor(brows0, c["TB0"], cz_ps, ALU.add)
            nc.vector.tensor_tensor(brows10, c["TB10"], cz_ps, ALU.add)
            nc.vector.tensor_tensor(brows, c["TB"], czb, ALU.add)

        psE_cm.__exit__(None, None, None)
        ps_cm = tc.tile_pool(name="ps", bufs=3, space="PSUM")
        ps = ps_cm.__enter__()

        a1w = state.tile([3, 20, 2, 128], F32R, tag="a1w")
        nc.sync.dma_start(out=a1w[0:2], in_=ins["a1top"])
        nc.sync.dma_start(out=a1w[2:3, 0].rearrange("a c d -> a (c d)"),
                          in_=brows[0:1])
        nc.sync.dma_start(out=a1w[2:3, STEPS].rearrange("a c d -> a (c d)"),
                          in_=brows[STEPS:STEPS + 1])
        nc.sync.dma_start(out=a1w[2:3, 1:STEPS].rearrange("a b c d -> a (b c d)"),
                          in_=brows[1:STEPS])
        nc.sync.dma_start(out=a1w[2:3, STEPS + 1:].rearrange("a b c d -> a (b c d)"),
                          in_=brows[STEPS + 1:])

        # ================= CNF =================
        if "cnf" not in PHASES:
            return
        # div-flow (h1sq/h2sq/ups/scr) is software-pipelined one chain behind
        # the main flow so its Pool/PE/DVE latency never blocks the Euler ring.
        from collections import deque
        pend = deque()

        def emit_div(i, nt, h1sq, h2sq):
            slot = i * NNT + nt
            sC = ps.tile([128, 2, NT], F32, tag="c", bufs=1)
            for mb in range(2):
                nc.tensor.matmul(sC[:, mb, :], c["nWuf8"][:, mb], h1sq,
                                 start=True, stop=False, perf_mode=DR)
                nc.tensor.matmul(sC[:, mb, :], c["ruf8"][:, mb, :],
                                 c["onesf8"][:, nt * NT:(nt + 1) * NT],
                                 start=False, stop=True)
            junk = work.tile([128, 2, NT], BF16, tag="junk", bufs=2)
            nc.vector.scalar_tensor_tensor(
                out=junk, in0=h2sq, scalar=1.0, in1=sC,
                op0=ALU.subtract, op1=ALU.mult,
                accum_out=divslots[:, slot:slot + 1])

        chains = [(i, nt) for i in range(STEPS) for nt in range(NNT)]

        def emit_a1(k):
            i, nt = chains[k]
            cur = i % 2
            sl = slice(nt * NT, (nt + 1) * NT)
            sA = ps.tile([128, 2, NT], F32, tag="s", bufs=2)
            sB = ps.tile([128, 2, NT], F32, tag="s", bufs=2)
            if i == 0:
                for mb in range(2):
                    mbs = slice(mb * 128, (mb + 1) * 128)
                    nc.tensor.matmul(sA[:, mb, :], a1w[0:2, 0, mb, :],
                                     stF[cur][0:2, sl], start=True, stop=False)
                    nc.tensor.matmul(sA[:, mb, :], brows0[:, mbs],
                                     ones1p[:, sl], start=False, stop=True)
                for mb in range(2):
                    mbs = slice(mb * 128, (mb + 1) * 128)
                    nc.tensor.matmul(sB[:, mb, :], a1w[0:2, STEPS, mb, :],
                                     stG[cur][0:2, sl], start=True, stop=False)
                    nc.tensor.matmul(sB[:, mb, :], brows10[:, mbs],
                                     ones1p[:, sl], start=False, stop=True)
            else:
                for mb in range(2):
                    nc.tensor.matmul(sA[:, mb, :], a1w[:, i, mb, :],
                                     stF[cur][:, sl], start=True, stop=True)
                for mb in range(2):
                    nc.tensor.matmul(sB[:, mb, :], a1w[:, STEPS + i, mb, :],
                                     stG[cur][:, sl], start=True, stop=True)
            return sA, sB

        nxt_slots = emit_a1(0)
        for k, (i, nt) in enumerate(chains):
            cur, nxt = i % 2, (i + 1) % 2
            sl = slice(nt * NT, (nt + 1) * NT)
            sA, sB = nxt_slots
            h1f = work.tile([128, 2, NT], FP8, tag="h1", bufs=8)
            nc.scalar.activation(h1f, sA, AF.Tanh)
            h1g = work.tile([128, 2, NT], FP8, tag="h1", bufs=8)
            nc.scalar.activation(h1g, sB, AF.Tanh)
            for mb in range(2):
                nc.tensor.matmul(sA[:, mb, :], c["W2f8"][:, mb], h1f,
                                 start=True, stop=True, perf_mode=DR)
            for mb in range(2):
                nc.tensor.matmul(sB[:, mb, :], c["W2f8"][:, mb], h1g,
                                 start=True, stop=True, perf_mode=DR)
            h2f = work.tile([128, 2, NT], FP8, tag="h2", bufs=8)
            nc.scalar.activation(h2f, sA, AF.Tanh)
            h2g = work.tile([128, 2, NT], FP8, tag="h2", bufs=8)
            nc.scalar.activation(h2g, sB, AF.Tanh)
            vf = ps.tile([16, NT], F32, tag="v", bufs=2)
            nc.tensor.matmul(vf, c["vWf8"], h2f, start=True, stop=True,
                             perf_mode=DR)
            nc.vector.scalar_tensor_tensor(
                out=stF[nxt][0:2, sl], in0=vf[0:2], scalar=DT,
                in1=stF[cur][0:2, sl], op0=ALU.mult, op1=ALU.add)
            if k + 1 < len(chains):
                nxt_slots = emit_a1(k + 1)
            vg = ps.tile([16, NT], F32, tag="v", bufs=2)
            nc.tensor.matmul(vg, c["vWf8"], h2g, start=True, stop=True,
                             perf_mode=DR)
            gout = raugA[0:2, sl] if i == STEPS - 1 else stG[nxt][0:2, sl]
            nc.vector.scalar_tensor_tensor(
                out=gout, in0=vg[0:2], scalar=-DT,
                in1=stG[cur][0:2, sl], op0=ALU.mult, op1=ALU.add)
            h1sq = work.tile([128, 2, NT], FP8, tag="hsq", bufs=6)
            nc.vector.tensor_tensor(h1sq, h1f, h1f, ALU.mult)
            h2sq = work.tile([128, 2, NT], FP8, tag="hsq", bufs=6)
            nc.gpsimd.tensor_tensor(h2sq, h2f, h2f, ALU.mult)
            pend.append((i, nt, h1sq, h2sq))
            if len(pend) > 2:
                emit_div(*pend.popleft())
        while pend:
            emit_div(*pend.popleft())

        divacc = small.tile([128, 1], F32, tag="divacc")
        nc.vector.tensor_reduce(divacc, divslots, axis=AX.X, op=ALU.add)
        nc.sync.dma_start(out=outs["o_div"], in_=divacc)

        sy2scr = work.tile([2, N], F32, tag="sy2scr", bufs=1, name="sy2scr")
        sy2 = small.tile([2, 1], F32, tag="sy2")
        nc.scalar.activation(sy2scr, stF[STEPS % 2][0:2], AF.Square,
                             bias=c["b3c"], accum_out=sy2)
        nc.sync.dma_start(out=outs["o_sy2"], in_=sy2)

        ps_cm.__exit__(None, None, None)

        # ================= chamfer =================
        if "cham" not in PHASES:
            return
        sqr = work.tile([2, N], F32R, tag="sqr", bufs=1, name="sqr")
        for nt in range(NNT):
            sl = slice(nt * NT, (nt + 1) * NT)
            nc.vector.scalar_tensor_tensor(out=sqr[:, sl], in0=raugA[0:2, sl],
                                           scalar=1.0, in1=raugA[0:2, sl],
                                           op0=ALU.mult, op1=ALU.mult)
            nc.sync.dma_start(out=raugA[2:4, sl], in_=sqr[:, sl])

        chAmin = small.tile([128, 16], F32, tag="chAmin")
        runmin = state.tile([128, N], BF16, tag="runmin")
        with tc.tile_pool(name="psD", bufs=2, space="PSUM") as psD, \
                tc.tile_pool(name="wkC", bufs=2) as wkC:
            for blk in range(16):
                bsl = slice(blk * 128, (blk + 1) * 128)
                Dp = psD.tile([128, N], F32, tag="D")
                for mt in range(NNT):
                    msl = slice(mt * NT, (mt + 1) * NT)
                    nc.tensor.matmul(Dp[:, msl], raugA[:, bsl], xnegA[:, msl],
                                     start=True, stop=True)
                Dc = wkC.tile([128, N], BF16, tag="Dc")
                nc.scalar.activation(Dc, Dp, AF.Copy)
                # A-side: one fused op -> elementwise min of halves + reduce
                t1 = wkC.tile([128, N // 2], BF16, tag="t1")
                nc.vector.tensor_tensor(t1, Dc[:, 0:N // 2], Dc[:, N // 2:N],
                                        ALU.min)
                t2 = wkC.tile([128, N // 4], BF16, tag="t2")
                nc.vector.tensor_tensor(t2, t1[:, 0:N // 4], t1[:, N // 4:N // 2],
                                        ALU.min)
                t3 = wkC.tile([128, N // 8], BF16, tag="t3")
                nc.vector.tensor_tensor(t3, t2[:, 0:N // 8], t2[:, N // 8:N // 4],
                                        ALU.min)
                nc.vector.tensor_reduce(chAmin[:, blk:blk + 1], t3,
                                        axis=AX.X, op=ALU.min)
                # B-side running min across blocks
                if blk == 0:
                    nc.vector.tensor_copy(runmin, Dc)
                else:
                    nc.vector.tensor_tensor(runmin, Dc, runmin, ALU.min)
        chs = small.tile([128, 16], F32, tag="chs")
        red = small.tile([128, 1], F32, tag="red")
        nc.vector.tensor_scalar_max(chAmin, chAmin, 0.0)
        nc.scalar.activation(chs, chAmin, AF.Sqrt)
        nc.vector.tensor_reduce(red, chs, axis=AX.X, op=ALU.add)
        nc.sync.dma_start(out=outs["o_chA"], in_=red)
        chBmin = small.tile([128, 16], F32, tag="chBmin")
        with tc.tile_pool(name="psT", bufs=1, space="PSUM") as psT:
            tp = psT.tile([128, 16, 128], BF16, tag="T")
            for k in range(16):
                nc.tensor.transpose(tp[:, k, :], runmin[:, k * 128:(k + 1) * 128],
                                    c["identb"])
            nc.vector.tensor_reduce(chBmin, tp, axis=AX.X, op=ALU.min)
        chsB = small.tile([128, 16], F32, tag="chsB")
        nc.vector.tensor_scalar_max(chsB, chBmin, 0.0)
        chs2 = small.tile([128, 16], F32, tag="chs2")
        red2 = small.tile([128, 1], F32, tag="red2")
        nc.scalar.activation(chs2, chsB, AF.Sqrt)
        nc.vector.tensor_reduce(red2, chs2, axis=AX.X, op=ALU.add)
        nc.sync.dma_start(out=outs["o_chB"], in_=red2)


_NC_CACHE = {}


def _get_nc():
    if "nc" not in _NC_CACHE:
        _NC_CACHE["nc"] = build_nc()
    return _NC_CACHE["nc"]


def kernel(**inputs):
    inputs = {k: np.asarray(v, dtype=np.float32) if np.asarray(v).dtype != np.int32
              else np.asarray(v) for k, v in inputs.items()}
    pre = host_precompute(inputs)
    nc = _get_nc()

    ones_row = np.ones((1, N), np.float32)
    in_maps = []
    for b in range(B):
        m = {k: v for k, v in pre.items()}
        m["xT3"] = np.ascontiguousarray(
            np.concatenate([inputs["x"][b].T, ones_row], 0), np.float32)
        m["nT3"] = np.ascontiguousarray(
            np.concatenate([inputs["noise"][b].T, ones_row], 0), np.float32)
        m["epsc"] = np.ascontiguousarray(inputs["eps"][b][:, None], np.float32)
        in_maps.append(m)

    res = run_bass_kernel_spmd(nc, in_maps, core_ids=list(range(B)))
    return combine(res.results, pre)


def combine(results, pre):
    S_logpy = 0.0
    S_logdet = 0.0
    prior = 0.0
    entropy = 0.0
    chamA = 0.0
    chamB = 0.0
    for r in results:
        S_logpy += -0.5 * float(r["o_sy2"].sum()) - N * LOG2PI
        # scr = (h2sq-1)*WU_SCALE*(ru - u) = -WU_SCALE*div per point
        S_logdet += DT * (-float(r["o_div"].sum()) / WU_SCALE)
        mu = r["o_mu"].astype(np.float64)
        lv = r["o_lv"].astype(np.float64)
        prior += 0.5 * float((mu ** 2 + np.exp(lv) - lv - 1.0).sum())
        entropy += -0.5 * float((lv + 1.0 + LOG2PI).sum())
        chamA += float(r["o_chA"].sum())
        chamB += float(r["o_chB"].sum())
    recon = -(S_logpy + S_logdet) / (B * N)
    prior /= B
    entropy /= B
    cham = chamA / (B * N) + chamB / (B * N)
    vol = max(0.0, S_logdet / (B * N) - 10.0)
    return np.float32(LAM_R * recon + LAM_P * prior + LAM_E * entropy
                      + LAM_C * cham + LAM_V * vol)
